# revision 1
# baseline (speedup 1.0000x reference)
"""EquivariantDense kernel for Trainium2 (8 NeuronCores, data-parallel over batch).

Math: with K = 4096, N = 4K, shift = K, the reference computes
    out[b, i*O4 + o] = sum_j sum_k w_{j+1}[b, o, k] * x[b, ((i+j)%4)*K + k]
i.e. per batch, 4 weight matrices (1024, 4096) each hit the 4 chunks of x.

Device mapping (per core = one batch):
  - PE matmul out[m,n] = sum_p lhsT[p,m] * rhs[p,n] contracts over partitions,
    so weights are staged (on host) transposed to (k, o) layout.
  - stationary lhsT = x-chunk tile (128 k-part, 4 roll-columns) -> tiny LDWEIGHTS
  - moving rhs = W^T tile (128 k-part, 512 o) fp32
  - accumulate all 128 k-blocks (4 j * 32 kb) into PSUM (4, 512) x 2 o-halves
  - weights stream HBM->SBUF as 32 contiguous 2 MiB DMAs (memory-bound regime,
    ~64 MiB/core at ~358 GB/s HBM/core roofline)
"""

import numpy as np

import concourse.mybir as mybir
import concourse.tile as tile
from concourse import bacc, bass_utils

B = 8
O4 = 1024
K = 4096
N = 4 * K  # 16384
NBLK = N // 128  # 128 global k-blocks of 128
KB2 = 4  # k-blocks per DMA tile
NT = NBLK // KB2  # 32 DMA tiles, 2 MiB each

_nc_cache = None


def _build_program(repeat=1):
    # repeat>1 builds the same body repeated back-to-back; used only for
    # timing measurements (dispatch-overhead-free per-iteration estimates)
    nc = bacc.Bacc()
    f32 = mybir.dt.float32
    xs_d = nc.dram_tensor("xstat", [128, NBLK * 4], f32, kind="ExternalInput")
    wt_d = nc.dram_tensor("wt", [NT, 128, KB2 * O4], f32, kind="ExternalInput")
    out_d = nc.dram_tensor("out", [4, O4], f32, kind="ExternalOutput")

    with tile.TileContext(nc) as tc:
        with (
            tc.tile_pool(name="xp", bufs=1) as xp,
            tc.tile_pool(name="wp", bufs=4) as wp,
            tc.tile_pool(name="pp", bufs=2, space="PSUM") as pp,
            tc.tile_pool(name="op", bufs=2) as op,
        ):
            xs = xp.tile([128, NBLK * 4], f32)
            # SWDGE: keeps the SP HWDGE ring free for the weight stream.
            # (Loading xs via the ACT HWDGE ring instead correlated with
            # NRT_EXEC_UNIT_UNRECOVERABLE crashes under concurrent
            # dual-ring DMA; SWDGE here has been stable across many runs.)
            nc.gpsimd.dma_start(xs[:], xs_d[:])
            for _rep in range(repeat):
                ps0 = pp.tile([4, 512], f32, tag="ps0")
                ps1 = pp.tile([4, 512], f32, tag="ps1")
                # Read tiles highest-address-first: reverse of the input
                # upload order, so if the memory system keeps recently
                # written lines warm, the single cold pass hits them first.
                # Order is otherwise irrelevant (PSUM accumulation commutes).
                for tidx, t in enumerate(reversed(range(NT))):
                    w_tile = wp.tile([128, KB2 * O4], f32, tag="w")
                    if tidx < NT - 1:
                        nc.sync.dma_start(w_tile[:], wt_d[t])
                    else:
                        # split the last-issued tile per k-block so the final
                        # matmuls chase the stream and the tail stays short;
                        # the final k-block splits again per o-half so the
                        # very last matmul waits on only 256 KiB
                        for kk in range(KB2 - 1):
                            nc.sync.dma_start(
                                w_tile[:, kk * O4 : (kk + 1) * O4],
                                wt_d[t, :, kk * O4 : (kk + 1) * O4],
                            )
                        kk = KB2 - 1
                        nc.sync.dma_start(
                            w_tile[:, kk * O4 : kk * O4 + 512],
                            wt_d[t, :, kk * O4 : kk * O4 + 512],
                        )
                        nc.sync.dma_start(
                            w_tile[:, kk * O4 + 512 : (kk + 1) * O4],
                            wt_d[t, :, kk * O4 + 512 : (kk + 1) * O4],
                        )
                    for kb2 in range(KB2):
                        g = t * KB2 + kb2
                        lhsT = xs[:, g * 4 : (g + 1) * 4]
                        first = tidx == 0 and kb2 == 0
                        last = tidx == NT - 1 and kb2 == KB2 - 1
                        nc.tensor.matmul(
                            ps0[:],
                            lhsT,
                            w_tile[:, kb2 * O4 : kb2 * O4 + 512],
                            start=first,
                            stop=last,
                        )
                        nc.tensor.matmul(
                            ps1[:],
                            lhsT,
                            w_tile[:, kb2 * O4 + 512 : (kb2 + 1) * O4],
                            start=first,
                            stop=last,
                        )
                ot = op.tile([4, O4], f32, tag="ot")
                nc.vector.tensor_copy(ot[:, 0:512], ps0[:])
                nc.scalar.copy(ot[:, 512:O4], ps1[:])
                nc.sync.dma_start(out_d[:], ot[:])
    nc.compile()
    return nc


def _get_program():
    global _nc_cache
    if _nc_cache is None:
        _nc_cache = _build_program()
    return _nc_cache


def prepare_inputs(x, w1, w2, w3, w4):
    """Host-side marshalling: shard over batch, transpose W to (k, o) tiles."""
    x = np.ascontiguousarray(np.asarray(x), dtype=np.float32)
    # Weight staging: W[b, j, o, k] -> Wh[b, t, p, kb2*O4 + o]
    # where k = (t*4 + kb2)*128 + p and j = (t*4 + kb2) // 32.
    W = np.stack(
        [np.asarray(w, dtype=np.float32) for w in (w1, w2, w3, w4)], axis=1
    )  # (B, 4, O4, K)
    W6 = W.reshape(B, 4, O4, 8, KB2, 128)  # k = tq*512 + kb2*128 + p
    Wh = np.ascontiguousarray(W6.transpose(0, 1, 3, 5, 4, 2)).reshape(
        B, NT, 128, KB2 * O4
    )

    # x staging: xs[b, p, g*4 + c] = x[b, ((c + g//32) % 4)*K + (g%32)*128 + p]
    cols = np.arange(NBLK * 4)
    g = cols // 4
    c = cols % 4
    j = g // 32
    kb = g % 32
    src_base = ((c + j) % 4) * K + kb * 128  # (512,)
    xs = x[:, src_base[None, :] + np.arange(128)[:, None]]  # (B, 128, 512)
    xs = np.ascontiguousarray(xs, dtype=np.float32)
    return xs, Wh


def run(x, w1, w2, w3, w4, trace=False, **kwargs):
    xs, Wh = prepare_inputs(x, w1, w2, w3, w4)
    nc = _get_program()
    in_maps = [{"xstat": xs[b], "wt": Wh[b]} for b in range(B)]
    res = bass_utils.run_bass_kernel_spmd(
        nc, in_maps, list(range(B)), trace=trace, **kwargs
    )
    out = np.stack(
        [res.results[b]["out"].reshape(4 * O4) for b in range(B)]
    ).astype(np.float32)
    return out, res


def kernel(x, w1, w2, w3, w4):
    out, _ = run(x, w1, w2, w3, w4)
    return out



# revision 2
# speedup vs baseline: 3.4046x; 3.4046x over previous
"""EquivariantDense kernel for Trainium2 (8 NeuronCores, data-parallel over batch).

Math: with K = 4096, N = 4K, shift = K, the reference computes
    out[b, i*O4 + o] = sum_j sum_k w_{j+1}[b, o, k] * x[b, ((i+j)%4)*K + k]
i.e. per batch, 4 weight matrices (1024, 4096) each hit the 4 chunks of x.

The problem is pure HBM-bandwidth-bound (weights are used exactly once), so
weights are shipped as bf16 (rounding rel-err ~1e-3, well under the 2e-2
gate), halving HBM traffic vs f32: ~33.5 MiB/core at ~370 GB/s/core.

Device mapping (per core = one batch):
  - PE matmul out[m,n] = sum_p lhsT[p,m] * rhs[p,n] contracts over partitions,
    so weights are staged (on host) transposed to (k, o) layout.
  - stationary lhsT = x-chunk tile (128 k-part, 4 roll-columns) bf16
  - moving rhs = W^T tile (128 k-part, 512 o) bf16
  - accumulate all 128 k-blocks (4 j * 32 kb) into PSUM (4, 512) x 2 o-halves
  - weights stream HBM->SBUF as 16 contiguous 2 MiB DMAs
"""

import numpy as np
import ml_dtypes

import concourse.mybir as mybir
import concourse.tile as tile
from concourse import bacc, bass_utils

B = 8
O4 = 1024
K = 4096
N = 4 * K  # 16384
NBLK = N // 128  # 128 global k-blocks of 128
KB2 = 8  # k-blocks per DMA tile
NT = NBLK // KB2  # 16 DMA tiles, 2 MiB each (bf16)

_nc_cache = None


def _build_program(repeat=1):
    # repeat>1 builds the same body repeated back-to-back; used only for
    # timing measurements (dispatch-overhead-free per-iteration estimates)
    nc = bacc.Bacc()
    f32 = mybir.dt.float32
    bf16 = mybir.dt.bfloat16
    xs_d = nc.dram_tensor("xstat", [128, NBLK * 4], bf16, kind="ExternalInput")
    wt_d = nc.dram_tensor("wt", [NT, 128, KB2 * O4], bf16, kind="ExternalInput")
    out_d = nc.dram_tensor("out", [4, O4], f32, kind="ExternalOutput")

    with tile.TileContext(nc) as tc:
        with (
            tc.tile_pool(name="xp", bufs=1) as xp,
            tc.tile_pool(name="wp", bufs=4) as wp,
            tc.tile_pool(name="pp", bufs=2, space="PSUM") as pp,
            tc.tile_pool(name="op", bufs=2) as op,
        ):
            xs = xp.tile([128, NBLK * 4], bf16)
            # SWDGE: keeps the SP HWDGE ring free for the weight stream.
            nc.gpsimd.dma_start(xs[:], xs_d[:])
            for _rep in range(repeat):
                ps0 = pp.tile([4, 512], f32, tag="ps0")
                ps1 = pp.tile([4, 512], f32, tag="ps1")
                for tidx, t in enumerate(reversed(range(NT))):
                    w_tile = wp.tile([128, KB2 * O4], bf16, tag="w")
                    if tidx < NT - 1:
                        nc.sync.dma_start(w_tile[:], wt_d[t])
                    else:
                        # split the last-issued tile per k-block so the final
                        # matmuls chase the stream and the tail stays short;
                        # the final k-block splits again per o-half so the
                        # very last matmul waits on only 128 KiB
                        for kk in range(KB2 - 1):
                            nc.sync.dma_start(
                                w_tile[:, kk * O4 : (kk + 1) * O4],
                                wt_d[t, :, kk * O4 : (kk + 1) * O4],
                            )
                        kk = KB2 - 1
                        nc.sync.dma_start(
                            w_tile[:, kk * O4 : kk * O4 + 512],
                            wt_d[t, :, kk * O4 : kk * O4 + 512],
                        )
                        nc.sync.dma_start(
                            w_tile[:, kk * O4 + 512 : (kk + 1) * O4],
                            wt_d[t, :, kk * O4 + 512 : (kk + 1) * O4],
                        )
                    for kb2 in range(KB2):
                        g = t * KB2 + kb2
                        lhsT = xs[:, g * 4 : (g + 1) * 4]
                        first = tidx == 0 and kb2 == 0
                        last = tidx == NT - 1 and kb2 == KB2 - 1
                        nc.tensor.matmul(
                            ps0[:],
                            lhsT,
                            w_tile[:, kb2 * O4 : kb2 * O4 + 512],
                            start=first,
                            stop=last,
                        )
                        nc.tensor.matmul(
                            ps1[:],
                            lhsT,
                            w_tile[:, kb2 * O4 + 512 : (kb2 + 1) * O4],
                            start=first,
                            stop=last,
                        )
                ot = op.tile([4, O4], f32, tag="ot")
                nc.vector.tensor_copy(ot[:, 0:512], ps0[:])
                nc.scalar.copy(ot[:, 512:O4], ps1[:])
                nc.sync.dma_start(out_d[:], ot[:])
    nc.compile()
    return nc


def _get_program():
    global _nc_cache
    if _nc_cache is None:
        _nc_cache = _build_program()
    return _nc_cache


def prepare_inputs(x, w1, w2, w3, w4):
    """Host-side marshalling: shard over batch, transpose W to (k, o) tiles,
    downcast to bf16."""
    x = np.ascontiguousarray(np.asarray(x), dtype=np.float32)
    # Weight staging: W[b, j, o, k] -> Wh[b, t, p, kb2*O4 + o]
    # where k = (t*KB2 + kb2)*128 + p and j = (t*KB2 + kb2) // 32.
    W = np.stack(
        [np.asarray(w, dtype=np.float32) for w in (w1, w2, w3, w4)], axis=1
    )  # (B, 4, O4, K)
    W6 = W.reshape(B, 4, O4, K // (KB2 * 128), KB2, 128)
    Wh = np.ascontiguousarray(
        W6.transpose(0, 1, 3, 5, 4, 2).astype(ml_dtypes.bfloat16)
    ).reshape(B, NT, 128, KB2 * O4)

    # x staging: xs[b, p, g*4 + c] = x[b, ((c + g//32) % 4)*K + (g%32)*128 + p]
    cols = np.arange(NBLK * 4)
    g = cols // 4
    c = cols % 4
    j = g // 32
    kb = g % 32
    src_base = ((c + j) % 4) * K + kb * 128  # (512,)
    xs = x[:, src_base[None, :] + np.arange(128)[:, None]]  # (B, 128, 512)
    xs = np.ascontiguousarray(xs.astype(ml_dtypes.bfloat16))
    return xs, Wh


def run(x, w1, w2, w3, w4, trace=False, **kwargs):
    xs, Wh = prepare_inputs(x, w1, w2, w3, w4)
    nc = _get_program()
    in_maps = [{"xstat": xs[b], "wt": Wh[b]} for b in range(B)]
    res = bass_utils.run_bass_kernel_spmd(
        nc, in_maps, list(range(B)), trace=trace, **kwargs
    )
    out = np.stack(
        [res.results[b]["out"].reshape(4 * O4) for b in range(B)]
    ).astype(np.float32)
    return out, res


def kernel(x, w1, w2, w3, w4):
    out, _ = run(x, w1, w2, w3, w4)
    return out


# revision 3
# speedup vs baseline: 7.0700x; 2.0766x over previous
"""EquivariantDense kernel for Trainium2 (8 NeuronCores, data-parallel over batch).

Math: with K = 4096, N = 4K, shift = K, the reference computes
    out[b, i*O4 + o] = sum_j sum_k w_{j+1}[b, o, k] * x[b, ((i+j)%4)*K + k]
i.e. per batch, 4 weight matrices (1024, 4096) each hit the 4 chunks of x.

Memory-bound problem (weights used exactly once), attacked on three fronts:
  1. Weights ship compressed: half the k-blocks as int8 (per-k-column scales
     folded into the stationary x on host -> no on-device dequant math; the
     int8->bf16 cast is exact), half as bf16. ~25 MiB/core on the wire.
  2. int8 tiles are cast to bf16 on-chip, split between DVE and ACT.
  3. PE runs two concurrent column-group streams (tile_position (0,0) and
     (0,64), separate PSUM banks), ~2.9x the single-stream moving rate.

Device mapping (per core = one batch):
  - stationary lhsT = x-chunk tile (128 k-part, 4 roll-columns) bf16,
    pre-scaled on host for int8 k-blocks
  - moving rhs = W^T tile (128 k-part, 512 o) bf16 (direct or converted)
  - accumulate all 128 k-blocks into PSUM bank0 rows 0-3 (o 0:512) and
    bank1 rows 64-67 (o 512:1024)
"""

import numpy as np
import ml_dtypes

import concourse.mybir as mybir
import concourse.tile as tile
from concourse import bacc, bass_utils

B = 8
O4 = 1024
K = 4096
N = 4 * K  # 16384
NBLK = N // 128  # 128 global k-blocks of 128
KB2 = 8  # k-blocks per tile
NT = NBLK // KB2  # 16 tiles
I8_TILES = (1, 3, 5, 7, 9, 11, 13, 14)  # tiles shipped as int8 (rest bf16)
N_I8 = len(I8_TILES)
N_BF = NT - N_I8
FD = KB2 * O4  # 8192 free-dim elements per tile

_nc_cache = None


def _tile_class():
    bf_idx, i8_idx = {}, {}
    for t in range(NT):
        if t in I8_TILES:
            i8_idx[t] = len(i8_idx)
        else:
            bf_idx[t] = len(bf_idx)
    return bf_idx, i8_idx


def _build_program(repeat=1):
    nc = bacc.Bacc()
    f32 = mybir.dt.float32
    bf16 = mybir.dt.bfloat16
    i8 = mybir.dt.int8
    bf_idx, i8_idx = _tile_class()
    xs_d = nc.dram_tensor("xstat", [128, NBLK * 4], bf16, kind="ExternalInput")
    wb_d = nc.dram_tensor("wb", [N_BF, 128, FD], bf16, kind="ExternalInput")
    wq_d = nc.dram_tensor("wq", [N_I8, 128, FD], i8, kind="ExternalInput")
    out_d = nc.dram_tensor("out", [8, 512], f32, kind="ExternalOutput")

    with tile.TileContext(nc) as tc:
        with (
            tc.tile_pool(name="xp", bufs=1) as xp,
            tc.tile_pool(name="wbp", bufs=3) as wbp,
            tc.tile_pool(name="wqp", bufs=3) as wqp,
            tc.tile_pool(name="wcp", bufs=3) as wcp,
            tc.tile_pool(name="pp", bufs=2, space="PSUM") as pp,
            tc.tile_pool(name="op", bufs=2) as op,
        ):
            xs = xp.tile([128, NBLK * 4], bf16)
            # SWDGE keeps the SP HWDGE ring free for the weight stream
            nc.gpsimd.dma_start(xs[:], xs_d[:])
            for _rep in range(repeat):
                ps0 = pp.tile([128, 512], f32, tag="ps0")
                ps1 = pp.tile([128, 512], f32, tag="ps1")
                for t in range(NT):
                    if t in i8_idx:
                        q_tile = wqp.tile([128, FD], i8, tag="wq")
                        nc.sync.dma_start(q_tile[:], wq_d[i8_idx[t]])
                        w_tile = wcp.tile([128, FD], bf16, tag="wc")
                        h = FD // 2
                        nc.vector.tensor_copy(w_tile[:, 0:h], q_tile[:, 0:h])
                        nc.scalar.copy(w_tile[:, h:FD], q_tile[:, h:FD])
                    else:
                        w_tile = wbp.tile([128, FD], bf16, tag="wb")
                        if t < NT - 1:
                            nc.sync.dma_start(w_tile[:], wb_d[bf_idx[t]])
                        else:
                            # split the last tile per k-block so the final
                            # matmuls chase the stream and the tail stays short
                            for kk in range(KB2):
                                nc.sync.dma_start(
                                    w_tile[:, kk * O4 : (kk + 1) * O4],
                                    wb_d[bf_idx[t], :, kk * O4 : (kk + 1) * O4],
                                )
                    for kb2 in range(KB2):
                        g = t * KB2 + kb2
                        lhsT = xs[:, g * 4 : (g + 1) * 4]
                        first = t == 0 and kb2 == 0
                        last = t == NT - 1 and kb2 == KB2 - 1
                        nc.tensor.matmul(
                            ps0[0:4, :],
                            lhsT,
                            w_tile[:, kb2 * O4 : kb2 * O4 + 512],
                            start=first,
                            stop=last,
                            tile_position=(0, 0),
                            skip_group_check=True,
                        )
                        nc.tensor.matmul(
                            ps1[64:68, :],
                            lhsT,
                            w_tile[:, kb2 * O4 + 512 : (kb2 + 1) * O4],
                            start=first,
                            stop=last,
                            tile_position=(0, 64),
                            skip_group_check=True,
                        )
                ot = op.tile([128, 512], f32, tag="ot")
                nc.vector.tensor_copy(ot[0:4, :], ps0[0:4, :])
                nc.scalar.copy(ot[64:68, :], ps1[64:68, :])
                nc.sync.dma_start(out_d[0:4, :], ot[0:4, :])
                nc.sync.dma_start(out_d[4:8, :], ot[64:68, :])
    nc.compile()
    return nc


def _get_program():
    global _nc_cache
    if _nc_cache is None:
        _nc_cache = _build_program()
    return _nc_cache


def prepare_inputs(x, w1, w2, w3, w4):
    """Host-side marshalling: shard over batch, transpose W to (k, o) layout,
    quantize int8-class tiles per k-column, fold scales into stationary x."""
    x = np.ascontiguousarray(np.asarray(x), dtype=np.float32)
    W = np.stack(
        [np.asarray(w, dtype=np.float32) for w in (w1, w2, w3, w4)], axis=1
    )  # (B, 4, O4, K)
    # Wt[b, g, p, o] = w_{j(g)}[b, o, kb(g)*128 + p],  g = t*KB2 + kb2,
    # j = g // 32, kb = g % 32   (k = kb*128 + p)
    W6 = W.reshape(B, 4, O4, K // (KB2 * 128), KB2, 128)  # j, o, tq, kb2, p
    Wt = np.ascontiguousarray(W6.transpose(0, 1, 3, 4, 5, 2)).reshape(
        B, NBLK, 128, O4
    )

    # per-(g, p) scale for int8-class k-blocks
    bf_idx, i8_idx = _tile_class()
    scale = np.ones((B, NBLK, 128), dtype=np.float32)
    Wq = np.empty((B, N_I8, 128, FD), dtype=np.int8)
    Wb = np.empty((B, N_BF, 128, FD), dtype=ml_dtypes.bfloat16)
    for t in range(NT):
        blk = Wt[:, t * KB2 : (t + 1) * KB2]  # (B, KB2, 128, O4)
        if t in i8_idx:
            s = np.abs(blk).max(axis=3) / 127.0  # (B, KB2, 128)
            s = np.maximum(s, 1e-30)
            q = np.rint(blk / s[..., None]).astype(np.int8)
            Wq[:, i8_idx[t]] = q.transpose(0, 2, 1, 3).reshape(B, 128, FD)
            scale[:, t * KB2 : (t + 1) * KB2] = s
        else:
            Wb[:, bf_idx[t]] = (
                blk.transpose(0, 2, 1, 3)
                .reshape(B, 128, FD)
                .astype(ml_dtypes.bfloat16)
            )

    # x staging: xs[b, p, g*4 + c] = scale[b,g,p] * x[b, ((c+j)%4)*K + kb*128 + p]
    cols = np.arange(NBLK * 4)
    g = cols // 4
    c = cols % 4
    j = g // 32
    kb = g % 32
    src_base = ((c + j) % 4) * K + kb * 128  # (512,)
    xs = x[:, src_base[None, :] + np.arange(128)[:, None]]  # (B, 128, 512)
    xs = xs * scale.transpose(0, 2, 1)[:, :, g]
    xs = np.ascontiguousarray(xs.astype(ml_dtypes.bfloat16))
    return xs, Wb, Wq


def run(x, w1, w2, w3, w4, trace=False, **kwargs):
    xs, Wb, Wq = prepare_inputs(x, w1, w2, w3, w4)
    nc = _get_program()
    in_maps = [
        {"xstat": xs[b], "wb": Wb[b], "wq": Wq[b]} for b in range(B)
    ]
    res = bass_utils.run_bass_kernel_spmd(
        nc, in_maps, list(range(B)), trace=trace, **kwargs
    )
    out = np.stack(
        [
            np.concatenate(
                [res.results[b]["out"][0:4], res.results[b]["out"][4:8]], axis=1
            ).reshape(4 * O4)
            for b in range(B)
        ]
    ).astype(np.float32)
    return out, res


def kernel(x, w1, w2, w3, w4):
    out, _ = run(x, w1, w2, w3, w4)
    return out
